# revision 1
# baseline (speedup 1.0000x reference)
"""Trainium2 Bass kernel: training-mode Decorrelated Batch Norm (ZCA
whitening via inverse matrix square root) for X[128, 64, 56, 56] fp32.

Strategy (8 NeuronCores, data-parallel over batch), v2:
  - Each core gets 16 batches packed on host into:
      XB [128, 25088] bf16: partition (g*64+c) holds channel c of batch
        group g — the whitening/apply operand.
      XTS [128, 196*130] fp8: per 128-sample chunk j the columns are
        [q0 data (64) | ones | q1 data (64) | ones], where the data cols
        are the 128x128 block transpose of XB (samples on partitions).
        The embedded ones columns make the channel sums fall out of the
        same PE matmuls that form the Gram — stats need ONLY this stream.
  - Load phase: stream XTS first (3.26 MB), then XB (6.42 MB). Two PE
    matmuls per chunk accumulate [G0+G1 | s0+s1] into one PSUM [64, 65]
    tile, so sigma*m and the channel sums are ready right after the XTS
    stream — the collective launches ~13 us in, overlapped with XB.
  - Collective: [64, 66] fp32 payload (sigma/m | mean | local trace
    share), AllReduce — or AllGather + local tree-sum (often cheaper:
    one wire phase instead of two).
  - Replicated per core: trace-normalized coupled Newton-Schulz
    (2 iterations; eigenvalues of sigma/c are within ~3% of 1, so the
    residual after 2 iterations is ~1e-6) for wm = sigma^(-1/2).
  - Apply: block-diagonal stationary [wm 0; 0 wm] (128x128 bf16) makes
    xn = wm @ x one full-width PE matmul per 448-col chunk; fused bias
    subtract on DVE/ScalarE during PSUM evacuation; bf16 stores.
"""

import sys

for _p in ("/opt/trn_rl_repo", "/root/.axon_site/_ro/trn_rl_repo"):
    if _p not in sys.path:
        sys.path.append(_p)

from contextlib import ExitStack

import numpy as np

import concourse.bacc as bacc
import concourse.mybir as mybir
import concourse.tile as tile
from concourse import bass_utils

F32 = mybir.dt.float32
BF16 = mybir.dt.bfloat16
FP8 = mybir.dt.float8e4
ALU = mybir.AluOpType
ACTF = mybir.ActivationFunctionType

N, C, H, W = 128, 64, 56, 56
HW = H * W                # 3136
NCORES = 8
NB = N // NCORES          # 16 batches per core
NG = NB // 2              # 8 images per partition group
MLOC = NG * HW            # 25088 free columns per core
MTOT = N * HW             # 401408 global sample count
EPS = 1e-3
TK = 128                  # gram chunk: samples per matmul
NCHUNK = MLOC // TK       # 196
CH = 130                  # XTS cols per chunk: 64 | ones | 64 | ones
MT = NCHUNK * CH          # 25480
AK = 448                  # apply matmul free-dim chunk (3136 = 7*448)
NS_ITERS = 2
OUT_BF16 = True           # store Y as bf16 (host upcasts); halves store traffic
TRNORM = 64.0             # Newton-Schulz normalization: c = trace / TRNORM

# XTS DMA chunk sizes in gram-chunk units: small priming chunks so the
# Gram starts early, tapered tail so the last chunk's matmul isn't
# waiting on a huge transfer.
XTS_GROUPS = [4, 8, 16, 32, 32, 32, 32, 24, 12, 4]
assert sum(XTS_GROUPS) == NCHUNK
XB_CHUNKS = [3136] * 8
assert sum(XB_CHUNKS) == MLOC


def build_module(reps: int = 1, collective="AG"):
    if collective is True:
        collective = "AR"
    elif collective is False:
        collective = "none"
    assert collective in ("AR", "AG", "RDB", "none")

    nc = bacc.Bacc(
        "TRN2", target_bir_lowering=False, debug=False, num_devices=NCORES
    )
    xb_d = nc.dram_tensor("XB", [128, MLOC], BF16, kind="ExternalInput")
    xt_d = nc.dram_tensor("XTS", [128, MT], FP8, kind="ExternalInput")
    id_d = nc.dram_tensor("IDENT", [128, 128], F32, kind="ExternalInput")
    y_d = nc.dram_tensor("Y", [128, MLOC], BF16 if OUT_BF16 else F32, kind="ExternalOutput")

    with tile.TileContext(nc) as tc, ExitStack() as ctx:
        const = ctx.enter_context(tc.tile_pool(name="const", bufs=1))
        xbp = ctx.enter_context(tc.tile_pool(name="xbp", bufs=1))
        xtp = ctx.enter_context(tc.tile_pool(name="xtp", bufs=1))
        stat = ctx.enter_context(tc.tile_pool(name="stat", bufs=2))
        smps = ctx.enter_context(tc.tile_pool(name="smps", bufs=2, space="PSUM"))
        ost = ctx.enter_context(tc.tile_pool(name="ost", bufs=3))
        dram = ctx.enter_context(tc.tile_pool(name="dram", bufs=2, space="DRAM"))

        # ---- constants (identity DMAs after the first XTS chunks below) ----
        ones = const.tile([128, 128], F32)
        nc.vector.memset(ones[:], 1.0)
        ident = const.tile([128, 128], F32)
        cdup = const.tile([64, 128], F32)
        id3 = const.tile([64, 64], F32)
        epsI = const.tile([64, 64], F32)
        invn2 = const.tile([64, 1], F32)
        nc.vector.memset(invn2[:], 1.0 / (TRNORM * MTOT))

        # remote-DMA-broadcast allgather state: two receive buffers
        # (alternating by rep parity), 7 slots of [128, 66] each, written
        # remotely by the XOR-delta peers. Same NEFF on every core, so the
        # compile-time SBUF addresses and semaphore numbers agree globally.
        if collective == "RDB":
            rdb_send = nc.alloc_semaphore("rdb_send")
            rdb_recv = nc.alloc_semaphore("rdb_recv")
            rcv_bufs = [
                const.tile([128, 66 * 7], F32, name=f"rdb_rcv{i}")
                for i in range(2)
            ]

        xbv = xb_d.ap()
        xtv = xt_d.ap()
        yv = y_d.ap()

        for _rep in range(reps):
            x_bf = xbp.tile([128, MLOC], BF16, tag="x_bf")
            xt_all = xtp.tile([128, MT], FP8, tag="xt_all")

            # ---- phase 1: stream XTS (all stats) then XB (apply operand) ----
            with ExitStack() as ph1:
                gps = ph1.enter_context(
                    tc.tile_pool(name="gps", bufs=1, space="PSUM")
                )
                g_ps = gps.tile([64, 65], F32, tag="g")

                o = 0
                for k, gsz in enumerate(XTS_GROUPS):
                    w = gsz * CH
                    nc.sync.dma_start(xt_all[:, o:o + w], xtv[:, o:o + w])
                    o += w
                    if k == 0 and _rep == 0:
                        nc.scalar.dma_start(ident[:], id_d.ap())
                        nc.scalar.dma_start(cdup[:, 0:64], id_d.ap()[0:64, 0:64])
                        nc.scalar.dma_start(cdup[:, 64:128], id_d.ap()[0:64, 0:64])
                        nc.vector.tensor_scalar_mul(
                            id3[:], ident[0:64, 0:64], 3.0
                        )
                        nc.vector.tensor_scalar_mul(
                            epsI[:], ident[0:64, 0:64], EPS
                        )
                o = 0
                for w in XB_CHUNKS:
                    nc.sync.dma_start(x_bf[:, o:o + w], xbv[:, o:o + w])
                    o += w

                # Gram + channel sums: per chunk, two matmuls accumulate
                # [Gq | sq] for both quadrants into one [64, 65] PSUM tile
                for j in range(NCHUNK):
                    b = j * CH
                    nc.tensor.matmul(
                        g_ps[:],
                        lhsT=xt_all[:, b:b + 64],
                        rhs=xt_all[:, b:b + 65],
                        start=(j == 0), stop=False,
                    )
                    nc.tensor.matmul(
                        g_ps[:],
                        lhsT=xt_all[:, b + 65:b + 129],
                        rhs=xt_all[:, b + 65:b + 130],
                        start=False, stop=(j == NCHUNK - 1),
                    )

                # ---- phase 2: pack stats [sigma*m | mean*m | trace] ----
                if collective == "RDB":
                    # stat_sb rotates through 2 buffers; before rewriting,
                    # the sends that read this buffer two reps ago must have
                    # drained (send sem += 112 per rep's 7 broadcasts).
                    if _rep >= 2:
                        nc.vector.wait_ge(rdb_send, 112 * (_rep - 1))
                    stat_sb = stat.tile([128, 66], F32, tag="stat_sb")
                    nc.vector.memset(stat_sb[64:128, :], 0.0)
                else:
                    stat_sb = stat.tile([64, 66], F32, tag="stat_sb")
                nc.vector.tensor_scalar_mul(
                    stat_sb[0:64, 0:65], g_ps[:], 1.0 / MTOT
                )
                # local trace share: tr(G_loc)/(m*TRNORM) + eps*C/(TRNORM*8)
                diagm = stat.tile([64, 64], F32, tag="diagm")
                nc.vector.tensor_tensor(
                    diagm[:], g_ps[:, 0:64], ident[0:64, 0:64], op=ALU.mult
                )
                diagc = stat.tile([64, 1], F32, tag="diagc")
                nc.vector.tensor_reduce(
                    diagc[:], diagm[:], axis=mybir.AxisListType.X, op=ALU.add
                )
                tr_ps = smps.tile([1, 1], F32, tag="sm")
                nc.tensor.matmul(
                    tr_ps[:], lhsT=diagc[:], rhs=invn2[:],
                    start=True, stop=True,
                )
                nc.vector.tensor_scalar(
                    stat_sb[0:1, 65:66], tr_ps[:],
                    EPS * C / (TRNORM * NCORES), None, op0=ALU.add,
                )

            statg = stat.tile([64, 66], F32, tag="statg")
            if collective == "RDB":
                rcv = rcv_bufs[_rep % 2]
                for d in range(1, NCORES):
                    rdests = [None] * 8
                    rdests[d] = (0, d)
                    nc.gpsimd.remote_dma_broadcast(
                        rcv[:, (d - 1) * 66:d * 66], stat_sb[:],
                        remote_sem=rdb_recv, local_sem=rdb_send,
                        rdests=rdests,
                    )
                nc.gpsimd.trigger_dma(count=None)
                # all 7 peers' payloads for this rep have landed when the
                # recv sem (incremented +2 per arriving broadcast) reaches
                # 14 * (rep + 1) cumulatively
                nc.vector.wait_ge(rdb_recv, 14 * (_rep + 1))
                a0 = stat.tile([64, 66 * 4], F32, tag="rda")
                for k in range(3):
                    nc.vector.tensor_tensor(
                        a0[:, k * 66:(k + 1) * 66],
                        rcv[0:64, (2 * k) * 66:(2 * k + 1) * 66],
                        rcv[0:64, (2 * k + 1) * 66:(2 * k + 2) * 66],
                        op=ALU.add,
                    )
                nc.vector.tensor_tensor(
                    a0[:, 198:264], rcv[0:64, 396:462], stat_sb[0:64, :],
                    op=ALU.add,
                )
                b0 = stat.tile([64, 66 * 2], F32, tag="sgb")
                for k in range(2):
                    nc.vector.tensor_tensor(
                        b0[:, k * 66:(k + 1) * 66],
                        a0[:, (2 * k) * 66:(2 * k + 1) * 66],
                        a0[:, (2 * k + 1) * 66:(2 * k + 2) * 66],
                        op=ALU.add,
                    )
                nc.vector.tensor_tensor(
                    statg[:], b0[:, 0:66], b0[:, 66:132], op=ALU.add
                )
            if collective in ("AR", "AG", "none"):
                cc_in = dram.tile([64, 66], F32, tag="cc_in")
                nc.scalar.dma_start(cc_in[:], stat_sb[0:64, :])
            if collective == "AR":
                cc_out = dram.tile([64, 66], F32, tag="cc_out", addr_space="Shared")
                nc.gpsimd.collective_compute(
                    "AllReduce", ALU.add,
                    replica_groups=[list(range(NCORES))],
                    ins=[cc_in.opt()], outs=[cc_out.opt()],
                )
                nc.scalar.dma_start(statg[:], cc_out[:])
            elif collective == "AG":
                cc_out = dram.tile(
                    [64 * NCORES, 66], F32, tag="cc_outg", addr_space="Shared"
                )
                nc.gpsimd.collective_compute(
                    "AllGather", ALU.bypass,
                    replica_groups=[list(range(NCORES))],
                    ins=[cc_in.opt()], outs=[cc_out.opt()],
                )
                sg = stat.tile([64, 66 * NCORES], F32, tag="sg")
                for k in range(NCORES):
                    nc.scalar.dma_start(
                        sg[:, k * 66:(k + 1) * 66],
                        cc_out[k * 64:(k + 1) * 64, :],
                    )
                # tree-sum the 8 shards
                a0 = stat.tile([64, 66 * 4], F32, tag="sga")
                for k in range(4):
                    nc.vector.tensor_tensor(
                        a0[:, k * 66:(k + 1) * 66],
                        sg[:, (2 * k) * 66:(2 * k + 1) * 66],
                        sg[:, (2 * k + 1) * 66:(2 * k + 2) * 66],
                        op=ALU.add,
                    )
                b0 = stat.tile([64, 66 * 2], F32, tag="sgb")
                for k in range(2):
                    nc.vector.tensor_tensor(
                        b0[:, k * 66:(k + 1) * 66],
                        a0[:, (2 * k) * 66:(2 * k + 1) * 66],
                        a0[:, (2 * k + 1) * 66:(2 * k + 2) * 66],
                        op=ALU.add,
                    )
                nc.vector.tensor_tensor(
                    statg[:], b0[:, 0:66], b0[:, 66:132], op=ALU.add
                )
            elif collective == "none":
                cc_out = dram.tile([64, 66], F32, tag="cc_out")
                nc.scalar.dma_start(cc_out[:], cc_in[:])
                nc.scalar.dma_start(statg[:], cc_out[:])

            # ---- phase 3: sigma, Newton-Schulz, whitening matrix ----
            mean_col = statg[:, 64:65]
            sigma = stat.tile([64, 64], F32, tag="sigma")
            nc.vector.tensor_tensor(
                sigma[:], statg[:, 0:64], epsI[:], op=ALU.add
            )
            icrc = stat.tile([1, 2], F32, tag="icrc")
            nc.vector.reciprocal(icrc[:, 0:1], statg[0:1, 65:66])
            nc.scalar.sqrt(icrc[:, 1:2], icrc[:, 0:1])
            bc_ps = smps.tile([128, 2], F32, tag="sm")
            nc.tensor.matmul(
                bc_ps[:], lhsT=ones[0:1, 0:128], rhs=icrc[:],
                start=True, stop=True,
            )
            bcast = stat.tile([128, 2], F32, tag="bcast")
            nc.vector.tensor_copy(bcast[:], bc_ps[:])
            ic64 = bcast[0:64, 0:1]
            rc128 = bcast[:, 1:2]

            yt = stat.tile([64, 64], F32, tag="nsY")
            nc.vector.tensor_scalar(
                yt[:], sigma[:], ic64, None, op0=ALU.mult
            )
            # iteration 1 specialized for Z0 = I: T = 3I - Y0,
            # Y1 = 0.5*Y0@T, Z1 = 0.5*T (no ZY / TZ matmuls needed)
            tt = stat.tile([64, 64], F32, tag="nsT")
            nc.vector.tensor_tensor(tt[:], id3[:], yt[:], op=ALU.subtract)
            p2 = smps.tile([64, 64], F32, tag="sm")
            nc.tensor.matmul(p2[:], lhsT=yt[:], rhs=tt[:], start=True, stop=True)
            yn = stat.tile([64, 64], F32, tag="nsY")
            nc.vector.tensor_scalar_mul(yn[:], p2[:], 0.5)
            yt = yn
            zt = stat.tile([64, 64], F32, tag="nsZ")
            nc.vector.tensor_scalar_mul(zt[:], tt[:], 0.5)
            for it in range(1, NS_ITERS):
                last = it == NS_ITERS - 1
                p1 = smps.tile([64, 64], F32, tag="sm")
                nc.tensor.matmul(p1[:], lhsT=zt[:], rhs=yt[:], start=True, stop=True)
                tt = stat.tile([64, 64], F32, tag="nsT")
                nc.vector.tensor_tensor(tt[:], id3[:], p1[:], op=ALU.subtract)
                if not last:
                    p2 = smps.tile([64, 64], F32, tag="sm")
                    nc.tensor.matmul(
                        p2[:], lhsT=yt[:], rhs=tt[:], start=True, stop=True
                    )
                p3 = smps.tile([64, 64], F32, tag="sm")
                nc.tensor.matmul(p3[:], lhsT=tt[:], rhs=zt[:], start=True, stop=True)
                if not last:
                    yn = stat.tile([64, 64], F32, tag="nsY")
                    nc.vector.tensor_scalar_mul(yn[:], p2[:], 0.5)
                    yt = yn
                zn = stat.tile([64, 64], F32, tag="nsZ")
                nc.vector.tensor_scalar_mul(zn[:], p3[:], 0.5)
                zt = zn

            # block-diag stationary [wm 0; 0 wm] bf16; negb = -wm @ mean
            ws_ps = smps.tile([128, 64], F32, tag="sm")
            nc.tensor.matmul(ws_ps[:], lhsT=cdup[:], rhs=zt[:], start=True, stop=True)
            wm_bd = stat.tile([128, 128], BF16, tag="wm_bd")
            nc.vector.memset(wm_bd[0:64, 64:128], 0.0)
            nc.vector.memset(wm_bd[64:128, 0:64], 0.0)
            nc.vector.tensor_scalar(
                wm_bd[0:64, 0:64], ws_ps[0:64, :], rc128[0:64, :], None,
                op0=ALU.mult,
            )
            nc.vector.tensor_scalar(
                wm_bd[64:128, 64:128], ws_ps[64:128, :], rc128[64:128, :], None,
                op0=ALU.mult,
            )
            b_ps = smps.tile([64, 1], F32, tag="sm")
            nc.tensor.matmul(
                b_ps[:], lhsT=zt[:], rhs=mean_col, start=True, stop=True
            )
            b64 = stat.tile([64, 1], F32, tag="b64")
            nc.vector.tensor_copy(b64[:], b_ps[:])
            bs_ps = smps.tile([128, 1], F32, tag="sm")
            nc.tensor.matmul(
                bs_ps[:], lhsT=cdup[:], rhs=b64[:], start=True, stop=True
            )
            negb = stat.tile([128, 1], F32, tag="negb")
            nc.vector.tensor_scalar(
                negb[:], bs_ps[:], rc128, -1.0, op0=ALU.mult, op1=ALU.mult
            )

            # ---- phase 4: whiten + store ----
            # first image split for an earlier store start
            otiles = [(0, 2 * AK, AK), (2 * AK, HW - 2 * AK, AK)]
            otiles += [(b * HW, HW, AK) for b in range(1, NG)]
            with ExitStack() as ph4:
                aps = ph4.enter_context(
                    tc.tile_pool(name="aps", bufs=4, space="PSUM")
                )
                ei = 0
                for (obase, owid, ak) in otiles:
                    ot = ost.tile([128, HW], BF16 if OUT_BF16 else F32, tag="ot")
                    for j in range(owid // ak):
                        po = aps.tile([128, AK], F32, tag="po")
                        off = obase + j * ak
                        nc.tensor.matmul(
                            po[:, 0:ak], lhsT=wm_bd[:],
                            rhs=x_bf[:, off:off + ak],
                            start=True, stop=True,
                        )
                        osl = ot[:, j * ak:(j + 1) * ak]
                        ei += 1
                        if ei % 2 == 0:
                            nc.vector.tensor_scalar(
                                osl, po[:, 0:ak], negb[:], None, op0=ALU.add
                            )
                        else:
                            nc.scalar.activation(
                                osl, po[:, 0:ak], ACTF.Identity,
                                bias=negb[:], scale=1.0,
                            )
                    nc.scalar.dma_start(
                        yv[:, obase:obase + owid], ot[:, 0:owid]
                    )
    nc.compile()
    return nc


_NC_CACHE: dict = {}


def _get_module(reps: int = 1, collective="AG"):
    key = (reps, collective)
    if key not in _NC_CACHE:
        _NC_CACHE[key] = build_module(reps, collective)
    return _NC_CACHE[key]


def pack_shard(Xc: np.ndarray) -> np.ndarray:
    """[16, 64, 56, 56] -> [128, 25088] with row (g*64+c), col (n*3136+hw)."""
    return np.ascontiguousarray(
        Xc.reshape(2, NG, C, HW).transpose(0, 2, 1, 3).reshape(128, MLOC)
    )


def unpack_shard(Yp: np.ndarray) -> np.ndarray:
    """Inverse of pack_shard."""
    return Yp.reshape(2, C, NG, HW).transpose(0, 2, 1, 3).reshape(NB, C, H, W)


def make_in_maps(X: np.ndarray):
    import ml_dtypes

    X = np.asarray(X, dtype=np.float32)
    assert X.shape == (N, C, H, W), X.shape
    ident = np.eye(128, dtype=np.float32)
    maps = []
    for i in range(NCORES):
        xp = pack_shard(X[i * NB:(i + 1) * NB])
        xb = xp.astype(ml_dtypes.bfloat16)
        # xt[p, j, r] = xp[r, j*128+p]: 128x128 block transpose
        xt = xp.reshape(128, NCHUNK, TK).transpose(2, 1, 0)
        xts = np.empty((128, NCHUNK, CH), dtype=np.float32)
        xts[:, :, 0:64] = xt[:, :, 0:64]
        xts[:, :, 64] = 1.0
        xts[:, :, 65:129] = xt[:, :, 64:128]
        xts[:, :, 129] = 1.0
        xts = np.ascontiguousarray(
            xts.reshape(128, MT).astype(ml_dtypes.float8_e4m3)
        )
        maps.append({"XB": np.ascontiguousarray(xb), "XTS": xts, "IDENT": ident})
    return maps


def kernel(X: np.ndarray) -> np.ndarray:
    nc = _get_module()
    in_maps = make_in_maps(X)
    res = bass_utils.run_bass_kernel_spmd(nc, in_maps, core_ids=list(range(NCORES)))
    return np.concatenate(
        [unpack_shard(np.asarray(r["Y"]).astype(np.float32)) for r in res.results],
        axis=0,
    )



# revision 2
# speedup vs baseline: 2.1463x; 2.1463x over previous
"""Trainium2 Bass kernel: training-mode Decorrelated Batch Norm (ZCA
whitening via inverse matrix square root) for X[128, 64, 56, 56] fp32.

Strategy (8 NeuronCores, data-parallel over batch), v2:
  - Each core gets 16 batches packed on host into:
      XB [128, 25088] bf16: partition (g*64+c) holds channel c of batch
        group g — the whitening/apply operand.
      XTS [128, 196*130] fp8: per 128-sample chunk j the columns are
        [q0 data (64) | ones | q1 data (64) | ones], where the data cols
        are the 128x128 block transpose of XB (samples on partitions).
        The embedded ones columns make the channel sums fall out of the
        same PE matmuls that form the Gram — stats need ONLY this stream.
  - Load phase: stream XTS first (3.26 MB), then XB (6.42 MB). Two PE
    matmuls per chunk accumulate [G0+G1 | s0+s1] into one PSUM [64, 65]
    tile, so sigma*m and the channel sums are ready right after the XTS
    stream — the collective launches ~13 us in, overlapped with XB.
  - Collective: [64, 66] fp32 payload (sigma/m | mean | local trace
    share), AllReduce — or AllGather + local tree-sum (often cheaper:
    one wire phase instead of two).
  - Replicated per core: trace-normalized coupled Newton-Schulz
    (2 iterations; eigenvalues of sigma/c are within ~3% of 1, so the
    residual after 2 iterations is ~1e-6) for wm = sigma^(-1/2).
  - Apply: block-diagonal stationary [wm 0; 0 wm] (128x128 bf16) makes
    xn = wm @ x one full-width PE matmul per 448-col chunk; fused bias
    subtract on DVE/ScalarE during PSUM evacuation; bf16 stores.
"""

import sys

for _p in ("/opt/trn_rl_repo", "/root/.axon_site/_ro/trn_rl_repo"):
    if _p not in sys.path:
        sys.path.append(_p)

from contextlib import ExitStack

import numpy as np

import concourse.bacc as bacc
import concourse.mybir as mybir
import concourse.tile as tile
from concourse import bass_utils

F32 = mybir.dt.float32
BF16 = mybir.dt.bfloat16
FP8 = mybir.dt.float8e4
ALU = mybir.AluOpType
ACTF = mybir.ActivationFunctionType

N, C, H, W = 128, 64, 56, 56
HW = H * W                # 3136
NCORES = 8
NB = N // NCORES          # 16 batches per core
NG = NB // 2              # 8 images per partition group
MLOC = NG * HW            # 25088 free columns per core
MTOT = N * HW             # 401408 global sample count
EPS = 1e-3
TK = 128                  # gram chunk: samples per matmul
NCHUNK = MLOC // TK       # 196
CH = 130                  # XTS cols per chunk: 64 | ones | 64 | ones
MT = NCHUNK * CH          # 25480
AK = 448                  # apply matmul free-dim chunk (3136 = 7*448)
NS_ITERS = 2
OUT_BF16 = True           # store Y as bf16 (host upcasts); halves store traffic
TRNORM = 64.0             # Newton-Schulz normalization: c = trace / TRNORM

# XTS DMA chunk sizes in gram-chunk units: small priming chunks so the
# Gram starts early, tapered tail so the last chunk's matmul isn't
# waiting on a huge transfer.
XTS_GROUPS = [4, 8, 16, 32, 32, 32, 32, 24, 12, 4]
assert sum(XTS_GROUPS) == NCHUNK
XB_CHUNKS = [3136] * 8
assert sum(XB_CHUNKS) == MLOC


def build_module(reps: int = 1, collective="AG"):
    if collective is True:
        collective = "AR"
    elif collective is False:
        collective = "none"
    assert collective in ("AR", "AG", "RDB", "none")

    nc = bacc.Bacc(
        "TRN2", target_bir_lowering=False, debug=False, num_devices=NCORES
    )
    xb_d = nc.dram_tensor("XB", [128, MLOC], BF16, kind="ExternalInput")
    xt_d = nc.dram_tensor("XTS", [128, MT], FP8, kind="ExternalInput")
    id_d = nc.dram_tensor("IDENT", [128, 128], F32, kind="ExternalInput")
    y_d = nc.dram_tensor("Y", [128, MLOC], BF16 if OUT_BF16 else F32, kind="ExternalOutput")

    with tile.TileContext(nc) as tc, ExitStack() as ctx:
        const = ctx.enter_context(tc.tile_pool(name="const", bufs=1))
        xbp = ctx.enter_context(tc.tile_pool(name="xbp", bufs=1))
        xtp = ctx.enter_context(tc.tile_pool(name="xtp", bufs=1))
        stat = ctx.enter_context(tc.tile_pool(name="stat", bufs=2))
        smps = ctx.enter_context(tc.tile_pool(name="smps", bufs=2, space="PSUM"))
        ost = ctx.enter_context(tc.tile_pool(name="ost", bufs=3))
        dram = ctx.enter_context(tc.tile_pool(name="dram", bufs=2, space="DRAM"))

        # ---- constants (identity DMAs after the first XTS chunks below) ----
        ones = const.tile([128, 128], F32)
        nc.vector.memset(ones[:], 1.0)
        ident = const.tile([128, 128], F32)
        cdup = const.tile([64, 128], F32)
        id3 = const.tile([64, 64], F32)
        epsI = const.tile([64, 64], F32)
        invn2 = const.tile([64, 1], F32)
        nc.vector.memset(invn2[:], 1.0 / (TRNORM * MTOT))

        # remote-DMA-broadcast allgather state: two receive buffers
        # (alternating by rep parity), 7 slots of [128, 66] each, written
        # remotely by the XOR-delta peers. Same NEFF on every core, so the
        # compile-time SBUF addresses and semaphore numbers agree globally.
        if collective == "RDB":
            rdb_send = nc.alloc_semaphore("rdb_send")
            rdb_recv = nc.alloc_semaphore("rdb_recv")
            rcv_bufs = [
                const.tile([128, 66 * 7], F32, name=f"rdb_rcv{i}")
                for i in range(2)
            ]

        xbv = xb_d.ap()
        xtv = xt_d.ap()
        yv = y_d.ap()

        for _rep in range(reps):
            x_bf = xbp.tile([128, MLOC], BF16, tag="x_bf")
            xt_all = xtp.tile([128, MT], FP8, tag="xt_all")

            # ---- phase 1: stream XTS (all stats) then XB (apply operand) ----
            with ExitStack() as ph1:
                gps = ph1.enter_context(
                    tc.tile_pool(name="gps", bufs=1, space="PSUM")
                )
                g_ps = gps.tile([64, 65], F32, tag="g")

                o = 0
                for k, gsz in enumerate(XTS_GROUPS):
                    w = gsz * CH
                    nc.sync.dma_start(xt_all[:, o:o + w], xtv[:, o:o + w])
                    o += w
                    if k == 0 and _rep == 0:
                        nc.scalar.dma_start(ident[:], id_d.ap())
                        nc.scalar.dma_start(cdup[:, 0:64], id_d.ap()[0:64, 0:64])
                        nc.scalar.dma_start(cdup[:, 64:128], id_d.ap()[0:64, 0:64])
                        nc.vector.tensor_scalar_mul(
                            id3[:], ident[0:64, 0:64], 3.0
                        )
                        nc.vector.tensor_scalar_mul(
                            epsI[:], ident[0:64, 0:64], EPS
                        )
                o = 0
                for w in XB_CHUNKS:
                    nc.sync.dma_start(x_bf[:, o:o + w], xbv[:, o:o + w])
                    o += w

                # Gram + channel sums: per chunk, two matmuls accumulate
                # [Gq | sq] for both quadrants into one [64, 65] PSUM tile
                for j in range(NCHUNK):
                    b = j * CH
                    nc.tensor.matmul(
                        g_ps[:],
                        lhsT=xt_all[:, b:b + 64],
                        rhs=xt_all[:, b:b + 65],
                        start=(j == 0), stop=False,
                    )
                    nc.tensor.matmul(
                        g_ps[:],
                        lhsT=xt_all[:, b + 65:b + 129],
                        rhs=xt_all[:, b + 65:b + 130],
                        start=False, stop=(j == NCHUNK - 1),
                    )

                # ---- phase 2: pack stats [sigma*m | mean*m | trace] ----
                if collective == "RDB":
                    # stat_sb rotates through 2 buffers; before rewriting,
                    # the sends that read this buffer two reps ago must have
                    # drained (send sem += 112 per rep's 7 broadcasts).
                    if _rep >= 2:
                        nc.vector.wait_ge(rdb_send, 112 * (_rep - 1))
                    stat_sb = stat.tile([128, 66], F32, tag="stat_sb")
                    nc.vector.memset(stat_sb[64:128, :], 0.0)
                else:
                    stat_sb = stat.tile([64, 66], F32, tag="stat_sb")
                    nc.vector.memset(stat_sb[:, 65:66], 0.0)
                nc.vector.tensor_scalar_mul(
                    stat_sb[0:64, 0:65], g_ps[:], 1.0 / MTOT
                )
                # local trace share: tr(G_loc)/(m*TRNORM) + eps*C/(TRNORM*8)
                diagm = stat.tile([64, 64], F32, tag="diagm")
                nc.vector.tensor_tensor(
                    diagm[:], g_ps[:, 0:64], ident[0:64, 0:64], op=ALU.mult
                )
                diagc = stat.tile([64, 1], F32, tag="diagc")
                nc.vector.tensor_reduce(
                    diagc[:], diagm[:], axis=mybir.AxisListType.X, op=ALU.add
                )
                tr_ps = smps.tile([1, 1], F32, tag="sm")
                nc.tensor.matmul(
                    tr_ps[:], lhsT=diagc[:], rhs=invn2[:],
                    start=True, stop=True,
                )
                nc.vector.tensor_scalar(
                    stat_sb[0:1, 65:66], tr_ps[:],
                    EPS * C / (TRNORM * NCORES), None, op0=ALU.add,
                )

            statg = stat.tile([64, 66], F32, tag="statg")
            if collective == "RDB":
                rcv = rcv_bufs[_rep % 2]
                for d in range(1, NCORES):
                    rdests = [None] * 8
                    rdests[d] = (0, d)
                    nc.gpsimd.remote_dma_broadcast(
                        rcv[:, (d - 1) * 66:d * 66], stat_sb[:],
                        remote_sem=rdb_recv, local_sem=rdb_send,
                        rdests=rdests,
                    )
                nc.gpsimd.trigger_dma(count=None)
                # all 7 peers' payloads for this rep have landed when the
                # recv sem (incremented +2 per arriving broadcast) reaches
                # 14 * (rep + 1) cumulatively
                nc.vector.wait_ge(rdb_recv, 14 * (_rep + 1))
                a0 = stat.tile([64, 66 * 4], F32, tag="rda")
                for k in range(3):
                    nc.vector.tensor_tensor(
                        a0[:, k * 66:(k + 1) * 66],
                        rcv[0:64, (2 * k) * 66:(2 * k + 1) * 66],
                        rcv[0:64, (2 * k + 1) * 66:(2 * k + 2) * 66],
                        op=ALU.add,
                    )
                nc.vector.tensor_tensor(
                    a0[:, 198:264], rcv[0:64, 396:462], stat_sb[0:64, :],
                    op=ALU.add,
                )
                b0 = stat.tile([64, 66 * 2], F32, tag="sgb")
                for k in range(2):
                    nc.vector.tensor_tensor(
                        b0[:, k * 66:(k + 1) * 66],
                        a0[:, (2 * k) * 66:(2 * k + 1) * 66],
                        a0[:, (2 * k + 1) * 66:(2 * k + 2) * 66],
                        op=ALU.add,
                    )
                nc.vector.tensor_tensor(
                    statg[:], b0[:, 0:66], b0[:, 66:132], op=ALU.add
                )
            if collective in ("AR", "AG", "none"):
                cc_in = dram.tile([64, 66], F32, tag="cc_in")
                nc.scalar.dma_start(cc_in[:], stat_sb[0:64, :])
            if collective == "AR":
                cc_out = dram.tile([64, 66], F32, tag="cc_out", addr_space="Shared")
                nc.gpsimd.collective_compute(
                    "AllReduce", ALU.add,
                    replica_groups=[list(range(NCORES))],
                    ins=[cc_in.opt()], outs=[cc_out.opt()],
                )
                nc.scalar.dma_start(statg[:], cc_out[:])
            elif collective == "AG":
                cc_out = dram.tile(
                    [64 * NCORES, 66], F32, tag="cc_outg", addr_space="Shared"
                )
                nc.gpsimd.collective_compute(
                    "AllGather", ALU.bypass,
                    replica_groups=[list(range(NCORES))],
                    ins=[cc_in.opt()], outs=[cc_out.opt()],
                )
                sg = stat.tile([64, 66 * NCORES], F32, tag="sg")
                for k in range(NCORES):
                    nc.scalar.dma_start(
                        sg[:, k * 66:(k + 1) * 66],
                        cc_out[k * 64:(k + 1) * 64, :],
                    )
                # tree-sum the 8 shards
                a0 = stat.tile([64, 66 * 4], F32, tag="sga")
                for k in range(4):
                    nc.vector.tensor_tensor(
                        a0[:, k * 66:(k + 1) * 66],
                        sg[:, (2 * k) * 66:(2 * k + 1) * 66],
                        sg[:, (2 * k + 1) * 66:(2 * k + 2) * 66],
                        op=ALU.add,
                    )
                b0 = stat.tile([64, 66 * 2], F32, tag="sgb")
                for k in range(2):
                    nc.vector.tensor_tensor(
                        b0[:, k * 66:(k + 1) * 66],
                        a0[:, (2 * k) * 66:(2 * k + 1) * 66],
                        a0[:, (2 * k + 1) * 66:(2 * k + 2) * 66],
                        op=ALU.add,
                    )
                nc.vector.tensor_tensor(
                    statg[:], b0[:, 0:66], b0[:, 66:132], op=ALU.add
                )
            elif collective == "none":
                cc_out = dram.tile([64, 66], F32, tag="cc_out")
                nc.scalar.dma_start(cc_out[:], cc_in[:])
                nc.scalar.dma_start(statg[:], cc_out[:])

            # ---- phase 3: sigma, Newton-Schulz, whitening matrix ----
            mean_col = statg[:, 64:65]
            sigma = stat.tile([64, 64], F32, tag="sigma")
            nc.vector.tensor_tensor(
                sigma[:], statg[:, 0:64], epsI[:], op=ALU.add
            )
            icrc = stat.tile([1, 2], F32, tag="icrc")
            nc.vector.reciprocal(icrc[:, 0:1], statg[0:1, 65:66])
            nc.scalar.sqrt(icrc[:, 1:2], icrc[:, 0:1])
            bc_ps = smps.tile([128, 2], F32, tag="sm")
            nc.tensor.matmul(
                bc_ps[:], lhsT=ones[0:1, 0:128], rhs=icrc[:],
                start=True, stop=True,
            )
            bcast = stat.tile([128, 2], F32, tag="bcast")
            nc.vector.tensor_copy(bcast[:], bc_ps[:])
            ic64 = bcast[0:64, 0:1]
            rc128 = bcast[:, 1:2]

            yt = stat.tile([64, 64], F32, tag="nsY")
            nc.vector.tensor_scalar(
                yt[:], sigma[:], ic64, None, op0=ALU.mult
            )
            # iteration 1 specialized for Z0 = I: T = 3I - Y0,
            # Y1 = 0.5*Y0@T, Z1 = 0.5*T (no ZY / TZ matmuls needed)
            tt = stat.tile([64, 64], F32, tag="nsT")
            nc.vector.tensor_tensor(tt[:], id3[:], yt[:], op=ALU.subtract)
            p2 = smps.tile([64, 64], F32, tag="sm")
            nc.tensor.matmul(p2[:], lhsT=yt[:], rhs=tt[:], start=True, stop=True)
            yn = stat.tile([64, 64], F32, tag="nsY")
            nc.vector.tensor_scalar_mul(yn[:], p2[:], 0.5)
            yt = yn
            zt = stat.tile([64, 64], F32, tag="nsZ")
            nc.vector.tensor_scalar_mul(zt[:], tt[:], 0.5)
            for it in range(1, NS_ITERS):
                last = it == NS_ITERS - 1
                p1 = smps.tile([64, 64], F32, tag="sm")
                nc.tensor.matmul(p1[:], lhsT=zt[:], rhs=yt[:], start=True, stop=True)
                tt = stat.tile([64, 64], F32, tag="nsT")
                nc.vector.tensor_tensor(tt[:], id3[:], p1[:], op=ALU.subtract)
                if not last:
                    p2 = smps.tile([64, 64], F32, tag="sm")
                    nc.tensor.matmul(
                        p2[:], lhsT=yt[:], rhs=tt[:], start=True, stop=True
                    )
                p3 = smps.tile([64, 64], F32, tag="sm")
                nc.tensor.matmul(p3[:], lhsT=tt[:], rhs=zt[:], start=True, stop=True)
                if not last:
                    yn = stat.tile([64, 64], F32, tag="nsY")
                    nc.vector.tensor_scalar_mul(yn[:], p2[:], 0.5)
                    yt = yn
                zn = stat.tile([64, 64], F32, tag="nsZ")
                nc.vector.tensor_scalar_mul(zn[:], p3[:], 0.5)
                zt = zn

            # block-diag stationary [wm 0; 0 wm] bf16; negb = -wm @ mean
            ws_ps = smps.tile([128, 64], F32, tag="sm")
            nc.tensor.matmul(ws_ps[:], lhsT=cdup[:], rhs=zt[:], start=True, stop=True)
            wm_bd = stat.tile([128, 128], BF16, tag="wm_bd")
            nc.vector.memset(wm_bd[0:64, 64:128], 0.0)
            nc.vector.memset(wm_bd[64:128, 0:64], 0.0)
            nc.vector.tensor_scalar(
                wm_bd[0:64, 0:64], ws_ps[0:64, :], rc128[0:64, :], None,
                op0=ALU.mult,
            )
            nc.vector.tensor_scalar(
                wm_bd[64:128, 64:128], ws_ps[64:128, :], rc128[64:128, :], None,
                op0=ALU.mult,
            )
            b_ps = smps.tile([64, 1], F32, tag="sm")
            nc.tensor.matmul(
                b_ps[:], lhsT=zt[:], rhs=mean_col, start=True, stop=True
            )
            b64 = stat.tile([64, 1], F32, tag="b64")
            nc.vector.tensor_copy(b64[:], b_ps[:])
            bs_ps = smps.tile([128, 1], F32, tag="sm")
            nc.tensor.matmul(
                bs_ps[:], lhsT=cdup[:], rhs=b64[:], start=True, stop=True
            )
            negb = stat.tile([128, 1], F32, tag="negb")
            nc.vector.tensor_scalar(
                negb[:], bs_ps[:], rc128, -1.0, op0=ALU.mult, op1=ALU.mult
            )

            # ---- phase 4: whiten + store ----
            # first image split for an earlier store start
            otiles = [(0, 2 * AK, AK), (2 * AK, HW - 2 * AK, AK)]
            otiles += [(b * HW, HW, AK) for b in range(1, NG)]
            with ExitStack() as ph4:
                aps = ph4.enter_context(
                    tc.tile_pool(name="aps", bufs=4, space="PSUM")
                )
                ei = 0
                for (obase, owid, ak) in otiles:
                    ot = ost.tile([128, HW], BF16 if OUT_BF16 else F32, tag="ot")
                    for j in range(owid // ak):
                        po = aps.tile([128, AK], F32, tag="po")
                        off = obase + j * ak
                        nc.tensor.matmul(
                            po[:, 0:ak], lhsT=wm_bd[:],
                            rhs=x_bf[:, off:off + ak],
                            start=True, stop=True,
                        )
                        osl = ot[:, j * ak:(j + 1) * ak]
                        ei += 1
                        if ei % 2 == 0:
                            nc.vector.tensor_scalar(
                                osl, po[:, 0:ak], negb[:], None, op0=ALU.add
                            )
                        else:
                            nc.scalar.activation(
                                osl, po[:, 0:ak], ACTF.Identity,
                                bias=negb[:], scale=1.0,
                            )
                    nc.scalar.dma_start(
                        yv[:, obase:obase + owid], ot[:, 0:owid]
                    )
    nc.compile()
    return nc


_NC_CACHE: dict = {}


def _get_module(reps: int = 1, collective="AG"):
    key = (reps, collective)
    if key not in _NC_CACHE:
        _NC_CACHE[key] = build_module(reps, collective)
    return _NC_CACHE[key]


def pack_shard(Xc: np.ndarray) -> np.ndarray:
    """[16, 64, 56, 56] -> [128, 25088] with row (g*64+c), col (n*3136+hw)."""
    return np.ascontiguousarray(
        Xc.reshape(2, NG, C, HW).transpose(0, 2, 1, 3).reshape(128, MLOC)
    )


def unpack_shard(Yp: np.ndarray) -> np.ndarray:
    """Inverse of pack_shard."""
    return Yp.reshape(2, C, NG, HW).transpose(0, 2, 1, 3).reshape(NB, C, H, W)


def make_in_maps(X: np.ndarray):
    import ml_dtypes

    X = np.asarray(X, dtype=np.float32)
    assert X.shape == (N, C, H, W), X.shape
    ident = np.eye(128, dtype=np.float32)
    maps = []
    for i in range(NCORES):
        xp = pack_shard(X[i * NB:(i + 1) * NB])
        xb = xp.astype(ml_dtypes.bfloat16)
        # xt[p, j, r] = xp[r, j*128+p]: 128x128 block transpose
        xt = xp.reshape(128, NCHUNK, TK).transpose(2, 1, 0)
        xts = np.empty((128, NCHUNK, CH), dtype=np.float32)
        xts[:, :, 0:64] = xt[:, :, 0:64]
        xts[:, :, 64] = 1.0
        xts[:, :, 65:129] = xt[:, :, 64:128]
        xts[:, :, 129] = 1.0
        xts = np.ascontiguousarray(
            xts.reshape(128, MT).astype(ml_dtypes.float8_e4m3)
        )
        maps.append({"XB": np.ascontiguousarray(xb), "XTS": xts, "IDENT": ident})
    return maps


def kernel(X: np.ndarray) -> np.ndarray:
    nc = _get_module()
    in_maps = make_in_maps(X)
    res = bass_utils.run_bass_kernel_spmd(nc, in_maps, core_ids=list(range(NCORES)))
    return np.concatenate(
        [unpack_shard(np.asarray(r["Y"]).astype(np.float32)) for r in res.results],
        axis=0,
    )

